# revision 2
# baseline (speedup 1.0000x reference)
"""Causal multi-head self-attention (RoPE) Trainium2 Bass kernel, 8-way
head-parallel.

Sharding: 16 heads / 8 cores = 2 heads per core (tensor parallel). Each core
receives the full (pre-transposed) activation matrix plus its head-slice of
w_qkv (with the RoPE interleave->half-split permutation folded into the
weight rows) and its 128-column slice of w_out. Each core computes a full
[8192, 1024] partial of the output projection; the host sums the 8 partials
(the all-reduce equivalent).

All matmul operands are pre-encoded on the host into the PE's FP32R format
(fp32 with the low 12 mantissa bits rounded away, RNE) so no on-chip
rounding passes are needed.

Per core:
  qkv^T = W @ x^T                 (M=384: Q,K,V rows; K accumulated in PSUM)
  u_sw  = SWAP @ u                (PE, 128x128 pair-swap matrix)
  rot   = u * cosA + u_sw * sinS  (RoPE on DVE; heads unstacked to base-0)
  S^T   = rot_k^T rot_q per 128-key tile; exp on ACT (scale=1/8) -> f32r
  causal: variable-N matmuls skip below-diagonal tiles; gpsimd affine_select
          zeroes the strict lower triangle of diagonal blocks
  PV    = [V | 1]^T @ E^T         (ones column gives the softmax denominator)
  out^T = PV[0:64] * recip(bcast(PV[64]))   (PE ones-row broadcast + DVE)
  y     = out^T^T @ w_out_slice^T (K=128, single matmul per tile)
"""
import os
import sys

for _p in ("/opt/trn_rl_repo", "/root/.axon_site/_ro/trn_rl_repo"):
    if os.path.isdir(_p) and _p not in sys.path:
        sys.path.insert(0, _p)

import numpy as np

B, S, D_MODEL, N_HEADS, D_HEAD = 4, 2048, 1024, 16, 64
N_CORES, H_PER = 8, 2
THETA = 10000.0
BS = B * S
KC = D_MODEL // 128             # 8 contraction chunks
NQ = S // 512                   # 4 query chunks per batch
NK = S // 128                   # 16 key tiles per batch

_PERM = np.concatenate([np.arange(0, 64, 2), np.arange(1, 64, 2)])
_INVF = THETA ** (-np.arange(32) * 2.0 / 64)

_cached = {}
TRACE = False            # set True to capture an NTFF profile on the next run
LAST_EXEC_NS = None      # max-core HW exec time of the last traced run
LAST_TRACE_PATH = None


def f32r_encode(a):
    """Round fp32 to the PE's FP32R format: RNE drop of low 12 mantissa bits."""
    xb = np.ascontiguousarray(a, np.float32).view(np.uint32).astype(np.uint64)
    low = xb & 0xFFF
    base = xb >> 12
    add = (low > 0x800) | ((low == 0x800) & ((base & 1) == 1))
    out = (((base + add) << 12) & 0xFFFFFFFF).astype(np.uint32)
    return out.view(np.float32).reshape(a.shape)


def _host_tables():
    pos = np.arange(S, dtype=np.float64)
    ang = pos[None, :] * _INVF[:, None]
    cosb, sinb = np.cos(ang), np.sin(ang)
    cosA64 = np.concatenate([cosb, cosb], 0)
    sinS64 = np.concatenate([-sinb, sinb], 0)
    cosA = np.concatenate([cosA64, cosA64], 0).astype(np.float32)   # [128, S]
    sinS = np.concatenate([sinS64, sinS64], 0).astype(np.float32)
    sw64 = np.zeros((64, 64), np.float64)
    sw64[:32, 32:] = np.eye(32)
    sw64[32:, :32] = np.eye(32)
    SW = np.block([[sw64, np.zeros((64, 64))], [np.zeros((64, 64)), sw64]])
    return cosA, sinS, SW


def _host_prep(x, w_qkv, w_out):
    cosA, sinS, SW = _host_tables()
    xT_r = f32r_encode(x.reshape(BS, D_MODEL).T)
    swap_r = f32r_encode(SW.astype(np.float32))
    in_maps = []
    for c in range(N_CORES):
        h0, h1 = 2 * c, 2 * c + 1
        wq = np.concatenate([w_qkv[64 * h0:64 * h0 + 64][_PERM],
                             w_qkv[64 * h1:64 * h1 + 64][_PERM]], 0)
        wk = np.concatenate([w_qkv[1024 + 64 * h0:1024 + 64 * h0 + 64][_PERM],
                             w_qkv[1024 + 64 * h1:1024 + 64 * h1 + 64][_PERM]], 0)
        wv = np.concatenate([w_qkv[2048 + 64 * h0:2048 + 64 * h0 + 64],
                             w_qkv[2048 + 64 * h1:2048 + 64 * h1 + 64]], 0)
        w_all = np.concatenate([wq, wk, wv], 0)          # [384, 1024]
        in_maps.append({
            "xT": xT_r,
            "wqkvT": f32r_encode(w_all.T),
            "woutT": f32r_encode(w_out[:, 128 * c:128 * c + 128].T),
            "swapm": swap_r,
            "cosA": cosA,
            "sinS": sinS,
        })
    return in_maps


def _build_nc():
    import concourse.bacc as bacc
    import concourse.mybir as mybir
    from concourse import tile
    from concourse.masks import make_identity

    F32, F32R = mybir.dt.float32, mybir.dt.float32r
    AF = mybir.ActivationFunctionType
    ALU = mybir.AluOpType

    nc = bacc.Bacc("TRN2", target_bir_lowering=False, debug=False,
                   num_devices=N_CORES)
    xT_d = nc.dram_tensor("xT", [D_MODEL, BS], F32R, kind="ExternalInput")
    w_d = nc.dram_tensor("wqkvT", [D_MODEL, 384], F32R, kind="ExternalInput")
    wo_d = nc.dram_tensor("woutT", [128, D_MODEL], F32R, kind="ExternalInput")
    sw_d = nc.dram_tensor("swapm", [128, 128], F32R, kind="ExternalInput")
    cos_d = nc.dram_tensor("cosA", [128, S], F32, kind="ExternalInput")
    sin_d = nc.dram_tensor("sinS", [128, S], F32, kind="ExternalInput")
    y_d = nc.dram_tensor("y", [BS, D_MODEL], F32, kind="ExternalOutput")

    with tile.TileContext(nc) as tc:
        with tc.tile_pool(name="const", bufs=1) as const, \
             tc.tile_pool(name="xr", bufs=2) as xrp, \
             tc.tile_pool(name="ur", bufs=3) as urp, \
             tc.tile_pool(name="ropet", bufs=2) as ropet, \
             tc.tile_pool(name="rot", bufs=2) as rotp, \
             tc.tile_pool(name="vr", bufs=1) as vrp, \
             tc.tile_pool(name="vone", bufs=2) as vonep, \
             tc.tile_pool(name="ep", bufs=4) as ep, \
             tc.tile_pool(name="outp", bufs=2) as outp, \
             tc.tile_pool(name="nrm", bufs=2) as nrm, \
             tc.tile_pool(name="ysb", bufs=3) as ysb, \
             tc.tile_pool(name="ps", bufs=1, space="PSUM") as ps:

            # ---- constants -------------------------------------------------
            w_r = const.tile([128, KC, 384], F32R, tag="w")
            nc.sync.dma_start(
                w_r[:], w_d.ap().rearrange("(kc p) m -> p kc m", p=128))
            wo_r = const.tile([128, D_MODEL], F32R, tag="wo")
            nc.sync.dma_start(wo_r[:], wo_d.ap())
            swap_r = const.tile([128, 128], F32R, tag="swap")
            nc.sync.dma_start(swap_r[:], sw_d.ap())
            cosA = const.tile([128, S], F32, tag="cos")
            sinS = const.tile([128, S], F32, tag="sin")
            nc.sync.dma_start(cosA[:], cos_d.ap())
            nc.sync.dma_start(sinS[:], sin_d.ap())
            idf = const.tile([128, 128], F32, tag="idf")
            make_identity(nc, idf[:])
            id128_r = const.tile([128, 128], F32R, tag="idr")
            nc.vector.tensor_copy(id128_r[:], idf[:])
            onef = const.tile([128, 1], F32, tag="onef")
            nc.vector.memset(onef[:], 1.0)
            one_r = const.tile([128, 1], F32R, tag="oner")
            nc.vector.tensor_copy(one_r[:], onef[:])
            orow_f = const.tile([1, 64], F32, tag="orowf")
            nc.vector.memset(orow_f[:], 1.0)
            orow_r = const.tile([1, 64], F32R, tag="orow")
            nc.vector.tensor_copy(orow_r[:], orow_f[:])

            for b in range(B):
                c0 = b * S

                # ---- QKV projection + RoPE + V transpose -------------------
                rot_q = rotp.tile([64, 2 * S], F32R, tag="rq")
                rot_k = rotp.tile([64, 2 * S], F32R, tag="rk")
                v_r = vrp.tile([128, S], F32R, tag="vr")
                vone = vonep.tile([128, H_PER, NK, 65], F32R, tag="vone")

                for n in range(NQ):
                    nsl = slice(n * 512, (n + 1) * 512)
                    x_r = xrp.tile([128, KC, 512], F32R, tag="xr")
                    for half in range(2):
                        nc.sync.dma_start(
                            x_r[:, 4 * half:4 * half + 4, :],
                            xT_d.ap()[512 * half:512 * half + 512,
                                      c0 + n * 512:c0 + (n + 1) * 512]
                            .rearrange("(kc p) n -> p kc n", p=128))

                    # projection chains first (PE stays busy), swaps
                    # interleaved so PE never waits on the ACT evictions
                    pus, urs = [], []
                    for qk in range(2):        # 0: Q, 1: K
                        pu = ps.tile([128, 512], F32, tag="a", bufs=4)
                        for kc in range(KC):
                            nc.tensor.matmul(
                                pu[:], w_r[:, kc, 128 * qk:128 * qk + 128],
                                x_r[:, kc, :], start=(kc == 0),
                                stop=(kc == KC - 1))
                        u_r = urp.tile([128, 512], F32R, tag="ur")
                        nc.scalar.copy(u_r[:], pu[:])
                        pus.append(pu)
                        urs.append(u_r)
                    pvv = ps.tile([128, 512], F32, tag="v", bufs=2)
                    for kc in range(KC):
                        nc.tensor.matmul(pvv[:], w_r[:, kc, 256:384],
                                         x_r[:, kc, :], start=(kc == 0),
                                         stop=(kc == KC - 1))
                    nc.scalar.copy(v_r[:, n * 512:(n + 1) * 512], pvv[:])
                    for qk in range(2):
                        u_r = urs[qk]
                        psw = ps.tile([128, 512], F32, tag="a", bufs=4)
                        nc.tensor.matmul(psw[:], swap_r[:], u_r[:],
                                         start=True, stop=True)
                        t_sb = ropet.tile([128, 512], F32, tag="t")
                        nc.vector.tensor_mul(t_sb[:], psw[:], sinS[:, nsl])
                        m_sb = ropet.tile([128, 512], F32, tag="m")
                        nc.vector.tensor_mul(m_sb[:], u_r[:], cosA[:, nsl])
                        rot = rot_q if qk == 0 else rot_k
                        for hh in range(H_PER):
                            nc.vector.tensor_add(
                                rot[:, hh * S + n * 512: hh * S + (n + 1) * 512],
                                t_sb[64 * hh:64 * hh + 64, :],
                                m_sb[64 * hh:64 * hh + 64, :])

                for ki in range(NK):
                    pt = ps.tile([128, 128], F32R, tag="v", bufs=2)
                    nc.tensor.transpose(
                        pt[:], v_r[:, ki * 128:(ki + 1) * 128], id128_r[:])
                    for hh in range(H_PER):
                        nc.vector.tensor_copy(vone[:, hh, ki, :64],
                                              pt[:, 64 * hh:64 * hh + 64])
                        nc.gpsimd.tensor_copy(vone[:, hh, ki, 64:65], one_r[:])

                # ---- attention --------------------------------------------
                outT = outp.tile([128, S], F32R, tag="outT")
                for hh in range(H_PER):
                    for qc in range(NQ):
                        pv = ps.tile([65, 512], F32, tag="pv", bufs=2)
                        last_ki = 4 * qc + 3
                        pending = []   # (e_t, nc_cols, ki) awaiting their PV
                        for ki in range(last_ki + 1):
                            nc_cols = 512 - max(0, ki - 4 * qc) * 128
                            coff = max(qc * 512, ki * 128)
                            st = ps.tile([128, 512], F32, tag="a", bufs=4)
                            nc.tensor.matmul(
                                st[:, :nc_cols],
                                rot_k[:, hh * S + ki * 128: hh * S + (ki + 1) * 128],
                                rot_q[:, hh * S + coff: hh * S + coff + nc_cols],
                                start=True, stop=True)
                            e_t = ep.tile([128, 512], F32R, tag="e")
                            nc.scalar.activation(e_t[:, :nc_cols], st[:, :nc_cols],
                                                 AF.Exp, scale=0.125)
                            if ki >= 4 * qc:   # diagonal block: zero k > q
                                nc.gpsimd.affine_select(
                                    out=e_t[:, :128], in_=e_t[:, :128],
                                    compare_op=ALU.is_ge, fill=0.0,
                                    base=0, pattern=[[1, 128]],
                                    channel_multiplier=-1)
                            pending.append((e_t, nc_cols, ki))
                            if len(pending) > 1:
                                pe_t, pnc, pki = pending.pop(0)
                                nc.tensor.matmul(
                                    pv[:, 512 - pnc:], vone[:, hh, pki, :],
                                    pe_t[:, :pnc], start=(pki == 0),
                                    stop=(pki == last_ki))
                        for pe_t, pnc, pki in pending:
                            nc.tensor.matmul(
                                pv[:, 512 - pnc:], vone[:, hh, pki, :],
                                pe_t[:, :pnc], start=(pki == 0),
                                stop=(pki == last_ki))

                        den_r = nrm.tile([1, 512], F32R, tag="den")
                        nc.scalar.copy(den_r[:], pv[64:65, :])
                        pbc = ps.tile([64, 512], F32, tag="v", bufs=2)
                        nc.tensor.matmul(pbc[:], orow_r[:], den_r[:],
                                         start=True, stop=True)
                        rb = nrm.tile([64, 512], F32, tag="rb")
                        nc.vector.reciprocal(rb[:], pbc[:])
                        nc.vector.tensor_mul(
                            outT[64 * hh:64 * hh + 64, qc * 512:(qc + 1) * 512],
                            pv[:64, :], rb[:])

                # ---- output projection ------------------------------------
                for t in range(16):
                    for ec in range(2):
                        py = ps.tile([128, 512], F32, tag="pv", bufs=2)
                        nc.tensor.matmul(py[:],
                                         outT[:, t * 128:(t + 1) * 128],
                                         wo_r[:, ec * 512:(ec + 1) * 512],
                                         start=True, stop=True)
                        y_sb = ysb.tile([128, 512], F32, tag="ysb")
                        nc.vector.tensor_copy(y_sb[:], py[:])
                        nc.sync.dma_start(
                            y_d.ap()[c0 + t * 128: c0 + (t + 1) * 128,
                                     ec * 512:(ec + 1) * 512],
                            y_sb[:])
    nc.compile()
    return nc


def _get_nc():
    if "nc" not in _cached:
        _cached["nc"] = _build_nc()
    return _cached["nc"]


def kernel(x, w_qkv, w_out):
    from concourse.bass_utils import run_bass_kernel_spmd

    x = np.asarray(x, np.float32)
    w_qkv = np.asarray(w_qkv, np.float32)
    w_out = np.asarray(w_out, np.float32)
    in_maps = _host_prep(x, w_qkv, w_out)
    nc = _get_nc()
    res = run_bass_kernel_spmd(nc, in_maps, core_ids=list(range(N_CORES)),
                               trace=TRACE)
    global LAST_EXEC_NS, LAST_TRACE_PATH, LAST_RESULT
    LAST_RESULT = res
    if res.exec_time_ns is not None:
        LAST_EXEC_NS = res.exec_time_ns
        if res.instructions_and_trace:
            LAST_TRACE_PATH = res.instructions_and_trace[1]
    y = np.sum(np.stack([res.results[c]["y"] for c in range(N_CORES)]),
               axis=0, dtype=np.float64)
    return y.reshape(B, S, D_MODEL).astype(np.float32)



# revision 13
# speedup vs baseline: 1.5210x; 1.5210x over previous
"""Causal multi-head self-attention (RoPE) Trainium2 Bass kernel, 8-way
head-parallel, bf16 datapath.

Sharding: 16 heads / 8 cores = 2 heads per core (tensor parallel). Each core
receives the full (pre-transposed) activation matrix plus its head-slice of
w_qkv (with the RoPE interleave->half-split permutation folded into the
weight rows) and its 128-column slice of w_out. Each core computes a full
[8192, 1024] partial of the output projection; the host sums the 8 partials
(the all-reduce equivalent).

All matmul operands are bf16 (PSUM accumulation stays fp32), halving SBUF/HBM
traffic and DVE/ACT element passes vs the fp32r version.

Per core and batch:
  qkv^T = W @ x^T                 (Q,K,V chains; K accumulated in PSUM)
  u_sw  = SWAP @ u                (PE, 128x128 pair-swap matrix)
  rot   = u*cosA + u_sw*sinS      (DVE; both heads stay stacked [128, S])
  S^T   = rot_k^T rot_q, two 128-key tiles paired per PSUM [128,1024] group;
          one exp (ACT, scale=1/8) per pair -> bf16
  causal: variable-N matmuls skip above-diagonal tiles; gpsimd affine_select
          zeroes the strict lower triangle of diagonal blocks
  PV    = [V | 1]^T @ E^T         (ones column gives the softmax denominator)
  rec   = reciprocal_approx_fast(PV[64]) -> bf16; PE ones-row broadcast;
          outT = PV[0:64] * bcast(rec)  (DVE)
  y     = outT^T @ w_out_slice^T  (K=128; per-qc so it overlaps attention)
"""
import os
import sys

for _p in ("/opt/trn_rl_repo", "/root/.axon_site/_ro/trn_rl_repo"):
    if os.path.isdir(_p) and _p not in sys.path:
        sys.path.insert(0, _p)

import numpy as np
import ml_dtypes

BF = ml_dtypes.bfloat16

B, S, D_MODEL, N_HEADS, D_HEAD = 4, 2048, 1024, 16, 64
N_CORES, H_PER = 8, 2
THETA = 10000.0
BS = B * S
KC = D_MODEL // 128             # 8 contraction chunks
NQ = S // 512                   # 4 query chunks per batch
NK = S // 128                   # 16 key tiles per batch

_PERM = np.concatenate([np.arange(0, 64, 2), np.arange(1, 64, 2)])
_INVF = THETA ** (-np.arange(32) * 2.0 / 64)

_cached = {}
USE_BASE64 = True        # heads stacked in rot [128,S]; hh=1 matmuls at base 64
USE_PAIR = True          # pair two key tiles per PSUM group, one exp per pair
USE_APPROX = True        # reciprocal_approx_fast for softmax denominator
TRACE = False            # set True to capture an NTFF profile on the next run
LAST_EXEC_NS = None      # max-core HW exec time of the last traced run
LAST_TRACE_PATH = None
LAST_RESULT = None


def _host_tables():
    pos = np.arange(S, dtype=np.float64)
    ang = pos[None, :] * _INVF[:, None]
    cosb, sinb = np.cos(ang), np.sin(ang)
    cosA64 = np.concatenate([cosb, cosb], 0)
    sinS64 = np.concatenate([-sinb, sinb], 0)
    cosA = np.concatenate([cosA64, cosA64], 0).astype(BF)   # [128, S]
    sinS = np.concatenate([sinS64, sinS64], 0).astype(BF)
    sw64 = np.zeros((64, 64), np.float64)
    sw64[:32, 32:] = np.eye(32)
    sw64[32:, :32] = np.eye(32)
    SW = np.block([[sw64, np.zeros((64, 64))], [np.zeros((64, 64)), sw64]])
    return cosA, sinS, SW.astype(BF)


def _host_prep(x, w_qkv, w_out):
    cosA, sinS, SW = _host_tables()
    xT = np.ascontiguousarray(x.reshape(BS, D_MODEL).T).astype(BF)
    in_maps = []
    for c in range(N_CORES):
        h0, h1 = 2 * c, 2 * c + 1
        wq = np.concatenate([w_qkv[64 * h0:64 * h0 + 64][_PERM],
                             w_qkv[64 * h1:64 * h1 + 64][_PERM]], 0)
        wk = np.concatenate([w_qkv[1024 + 64 * h0:1024 + 64 * h0 + 64][_PERM],
                             w_qkv[1024 + 64 * h1:1024 + 64 * h1 + 64][_PERM]], 0)
        wv = np.concatenate([w_qkv[2048 + 64 * h0:2048 + 64 * h0 + 64],
                             w_qkv[2048 + 64 * h1:2048 + 64 * h1 + 64]], 0)
        w_all = np.concatenate([wq, wk, wv], 0)          # [384, 1024]
        in_maps.append({
            "xT": xT,
            "wqkvT": np.ascontiguousarray(w_all.T).astype(BF),
            "woutT": np.ascontiguousarray(
                w_out[:, 128 * c:128 * c + 128].T).astype(BF),
            "swapm": SW,
            "cosA": cosA,
            "sinS": sinS,
        })
    return in_maps


def _build_nc():
    import concourse.bacc as bacc
    import concourse.mybir as mybir
    from concourse import tile
    from concourse.masks import make_identity

    F32, BF16 = mybir.dt.float32, mybir.dt.bfloat16
    AF = mybir.ActivationFunctionType
    ALU = mybir.AluOpType

    nc = bacc.Bacc("TRN2", target_bir_lowering=False, debug=False,
                   num_devices=N_CORES)
    xT_d = nc.dram_tensor("xT", [D_MODEL, BS], BF16, kind="ExternalInput")
    w_d = nc.dram_tensor("wqkvT", [D_MODEL, 384], BF16, kind="ExternalInput")
    wo_d = nc.dram_tensor("woutT", [128, D_MODEL], BF16, kind="ExternalInput")
    sw_d = nc.dram_tensor("swapm", [128, 128], BF16, kind="ExternalInput")
    cos_d = nc.dram_tensor("cosA", [128, S], BF16, kind="ExternalInput")
    sin_d = nc.dram_tensor("sinS", [128, S], BF16, kind="ExternalInput")
    y_d = nc.dram_tensor("y", [BS, D_MODEL], BF16, kind="ExternalOutput")

    with tile.TileContext(nc) as tc:
        with tc.tile_pool(name="const", bufs=1) as const, \
             tc.tile_pool(name="xr", bufs=2) as xrp, \
             tc.tile_pool(name="ur", bufs=3) as urp, \
             tc.tile_pool(name="ropet", bufs=2) as ropet, \
             tc.tile_pool(name="rot", bufs=2) as rotp, \
             tc.tile_pool(name="vsb", bufs=2) as vsbp, \
             tc.tile_pool(name="vone", bufs=2) as vonep, \
             tc.tile_pool(name="ep", bufs=3) as ep, \
             tc.tile_pool(name="recp", bufs=2) as recp, \
             tc.tile_pool(name="outp", bufs=2) as outp, \
             tc.tile_pool(name="ysb", bufs=3) as ysb, \
             tc.tile_pool(name="ps", bufs=1, space="PSUM") as ps:

            # ---- constants -------------------------------------------------
            w_r = const.tile([128, KC, 384], BF16, tag="w")
            nc.sync.dma_start(
                w_r[:], w_d.ap().rearrange("(kc p) m -> p kc m", p=128))
            wo_r = const.tile([128, D_MODEL], BF16, tag="wo")
            nc.sync.dma_start(wo_r[:], wo_d.ap())
            swap_r = const.tile([128, 128], BF16, tag="swap")
            nc.sync.dma_start(swap_r[:], sw_d.ap())
            cosA = const.tile([128, S], BF16, tag="cos")
            sinS = const.tile([128, S], BF16, tag="sin")
            nc.sync.dma_start(cosA[:], cos_d.ap())
            nc.sync.dma_start(sinS[:], sin_d.ap())
            idf = const.tile([128, 128], F32, tag="idf")
            make_identity(nc, idf[:])
            id_b = const.tile([128, 128], BF16, tag="idb")
            nc.vector.tensor_copy(id_b[:], idf[:])
            onef = const.tile([128, 1], F32, tag="onef")
            nc.vector.memset(onef[:], 1.0)
            one_b = const.tile([128, 1], BF16, tag="oneb")
            nc.vector.tensor_copy(one_b[:], onef[:])
            orow_f = const.tile([1, 64], F32, tag="orowf")
            nc.vector.memset(orow_f[:], 1.0)
            orow_b = const.tile([1, 64], BF16, tag="orow")
            nc.vector.tensor_copy(orow_b[:], orow_f[:])

            for b in range(B):
                c0 = b * S

                # ---- QKV projection + RoPE ---------------------------------
                if USE_BASE64:
                    rot_q = rotp.tile([128, S], BF16, tag="rq")
                    rot_k = rotp.tile([128, S], BF16, tag="rk")
                else:
                    rot_q = rotp.tile([64, 2 * S], BF16, tag="rq")
                    rot_k = rotp.tile([64, 2 * S], BF16, tag="rk")

                def rsl(rot, hh, start, length):
                    if USE_BASE64:
                        return rot[64 * hh:64 * hh + 64, start:start + length]
                    return rot[:, hh * S + start:hh * S + start + length]
                v_sb = vsbp.tile([128, S], BF16, tag="vsb")
                vone = vonep.tile([128, H_PER, NK, 65], BF16, tag="vone")
                for hh in range(H_PER):
                    for ki in range(NK):
                        nc.gpsimd.tensor_copy(vone[:, hh, ki, 64:65], one_b[:])

                for n in range(NQ):
                    nsl = slice(n * 512, (n + 1) * 512)
                    x_r = xrp.tile([128, KC, 512], BF16, tag="xr")
                    for half in range(2):
                        nc.sync.dma_start(
                            x_r[:, 4 * half:4 * half + 4, :],
                            xT_d.ap()[512 * half:512 * half + 512,
                                      c0 + n * 512:c0 + (n + 1) * 512]
                            .rearrange("(kc p) n -> p kc n", p=128))

                    if USE_PAIR:
                        tqk = ps.tile([128, 1024], F32, tag="a", bufs=2)
                        halves = [tqk[:, :512], tqk[:, 512:]]
                    else:
                        t0 = ps.tile([128, 512], F32, tag="a", bufs=4)
                        t1 = ps.tile([128, 512], F32, tag="a", bufs=4)
                        halves = [t0[:], t1[:]]
                    for qk in range(2):        # 0: Q, 1: K
                        for kc in range(KC):
                            nc.tensor.matmul(
                                halves[qk],
                                w_r[:, kc, 128 * qk:128 * qk + 128],
                                x_r[:, kc, :], start=(kc == 0),
                                stop=(kc == KC - 1))
                    pvv = ps.tile([128, 512], F32, tag="v", bufs=2)
                    for kc in range(KC):
                        nc.tensor.matmul(pvv[:], w_r[:, kc, 256:384],
                                         x_r[:, kc, :], start=(kc == 0),
                                         stop=(kc == KC - 1))
                    uq = urp.tile([128, 512], BF16, tag="ur")
                    nc.scalar.copy(uq[:], halves[0])
                    uk = urp.tile([128, 512], BF16, tag="ur")
                    nc.scalar.copy(uk[:], halves[1])
                    nc.scalar.copy(v_sb[:, nsl], pvv[:])
                    if USE_PAIR:
                        tsw = ps.tile([128, 1024], F32, tag="a", bufs=2)
                        swh = [tsw[:, :512], tsw[:, 512:]]
                    else:
                        s0 = ps.tile([128, 512], F32, tag="a", bufs=4)
                        s1 = ps.tile([128, 512], F32, tag="a", bufs=4)
                        swh = [s0[:], s1[:]]
                    nc.tensor.matmul(swh[0], swap_r[:], uq[:],
                                     start=True, stop=True)
                    nc.tensor.matmul(swh[1], swap_r[:], uk[:],
                                     start=True, stop=True)
                    for qk, (u_sb, rot) in enumerate(((uq, rot_q),
                                                      (uk, rot_k))):
                        m_sb = ropet.tile([128, 512], BF16, tag="m")
                        nc.vector.tensor_mul(m_sb[:], u_sb[:], cosA[:, nsl])
                        t_sb = ropet.tile([128, 512], BF16, tag="t")
                        nc.vector.tensor_mul(
                            t_sb[:], swh[qk], sinS[:, nsl])
                        if USE_BASE64:
                            nc.vector.tensor_add(rot[:, nsl], m_sb[:],
                                                 t_sb[:])
                        else:
                            for hh in range(H_PER):
                                nc.vector.tensor_add(
                                    rsl(rot, hh, n * 512, 512),
                                    m_sb[64 * hh:64 * hh + 64, :],
                                    t_sb[64 * hh:64 * hh + 64, :])

                # ---- V transpose ------------------------------------------
                for ki in range(NK):
                    pt = ps.tile([128, 128], BF16, tag="v", bufs=2)
                    nc.tensor.transpose(
                        pt[:], v_sb[:, ki * 128:(ki + 1) * 128], id_b[:])
                    nc.vector.tensor_copy(
                        vone[:, :, ki, :64],
                        pt[:].rearrange("p (h d) -> p h d", h=2))

                # ---- attention + output projection, per query chunk -------
                outT = outp.tile([128, S], BF16, tag="outT")
                step = 2 if USE_PAIR else 1
                stw = 1024 if USE_PAIR else 512
                stb = 2 if USE_PAIR else 4
                for qc in range(NQ):
                    for hh in range(H_PER):
                        last_ki = 4 * qc + 3
                        pv = ps.tile([65, 512], F32, tag="pv", bufs=2)
                        # pair key tiles: (0,1), (2,3), ...
                        pend = []   # (e_t, [(ki, ncc, off)]) awaiting PV
                        for k0 in range(0, last_ki + 1, step):
                            group = []
                            off = 0
                            for ki in range(k0, k0 + step):
                                if ki > last_ki:
                                    break
                                ncc = 512 - max(0, ki - 4 * qc) * 128
                                group.append((ki, ncc, off))
                                off += ncc
                            stp = ps.tile([128, stw], F32, tag="a", bufs=stb)
                            e_t = ep.tile([128, stw], BF16, tag="e")
                            for ki, ncc, o in group:
                                coff = max(qc * 512, ki * 128)
                                nc.tensor.matmul(
                                    stp[:, o:o + ncc],
                                    rsl(rot_k, hh, ki * 128, 128),
                                    rsl(rot_q, hh, coff, ncc),
                                    start=True, stop=True)
                            nc.scalar.activation(e_t[:, :off], stp[:, :off],
                                                 AF.Exp, scale=0.125)
                            for ki, ncc, o in group:
                                if ki >= 4 * qc:   # diagonal block: zero k > q
                                    nc.gpsimd.affine_select(
                                        out=e_t[:, o:o + 128],
                                        in_=e_t[:, o:o + 128],
                                        compare_op=ALU.is_ge, fill=0.0,
                                        base=0, pattern=[[1, 128]],
                                        channel_multiplier=-1)
                            pend.append((e_t, group))
                            if len(pend) > 1:
                                pe_t, pgroup = pend.pop(0)
                                for ki, ncc, o in pgroup:
                                    nc.tensor.matmul(
                                        pv[:, 512 - ncc:],
                                        vone[:, hh, ki, :],
                                        pe_t[:, o:o + ncc],
                                        start=(ki == 0), stop=(ki == last_ki))
                        for pe_t, pgroup in pend:
                            for ki, ncc, o in pgroup:
                                nc.tensor.matmul(
                                    pv[:, 512 - ncc:], vone[:, hh, ki, :],
                                    pe_t[:, o:o + ncc],
                                    start=(ki == 0), stop=(ki == last_ki))

                        rec_f = recp.tile([1, 512], F32, tag="rec")
                        if USE_APPROX:
                            den_sb = recp.tile([1, 512], F32, tag="den")
                            nc.scalar.copy(den_sb[:], pv[64:65, :])
                            nc.vector.reciprocal_approx_fast(rec_f[:],
                                                             den_sb[:])
                        else:
                            nc.vector.reciprocal(rec_f[:], pv[64:65, :])
                        rec_b = recp.tile([1, 512], BF16, tag="recb")
                        nc.vector.tensor_copy(rec_b[:], rec_f[:])
                        pbc = ps.tile([64, 512], F32, tag="v", bufs=2)
                        nc.tensor.matmul(pbc[:], orow_b[:], rec_b[:],
                                         start=True, stop=True)
                        rb_sb = recp.tile([64, 512], BF16, tag="rb")
                        nc.scalar.copy(rb_sb[:], pbc[:])
                        nc.vector.tensor_mul(
                            outT[64 * hh:64 * hh + 64,
                                 qc * 512:(qc + 1) * 512],
                            pv[:64, :], rb_sb[:])

                    # ---- output projection for this qc's token tiles ------
                    for t in range(4 * qc, 4 * qc + 4):
                        for ec in range(2):
                            py = ps.tile([128, 512], F32, tag="pv", bufs=2)
                            nc.tensor.matmul(py[:],
                                             outT[:, t * 128:(t + 1) * 128],
                                             wo_r[:, ec * 512:(ec + 1) * 512],
                                             start=True, stop=True)
                            y_sb = ysb.tile([128, 512], BF16, tag="ysb")
                            nc.vector.tensor_copy(y_sb[:], py[:])
                            nc.sync.dma_start(
                                y_d.ap()[c0 + t * 128: c0 + (t + 1) * 128,
                                         ec * 512:(ec + 1) * 512],
                                y_sb[:])
    nc.compile()
    return nc


def _get_nc():
    if "nc" not in _cached:
        _cached["nc"] = _build_nc()
    return _cached["nc"]


def kernel(x, w_qkv, w_out):
    from concourse.bass_utils import run_bass_kernel_spmd

    x = np.asarray(x, np.float32)
    w_qkv = np.asarray(w_qkv, np.float32)
    w_out = np.asarray(w_out, np.float32)
    in_maps = _host_prep(x, w_qkv, w_out)
    nc = _get_nc()
    res = run_bass_kernel_spmd(nc, in_maps, core_ids=list(range(N_CORES)),
                               trace=TRACE)
    global LAST_EXEC_NS, LAST_TRACE_PATH, LAST_RESULT
    LAST_RESULT = res
    if res.exec_time_ns is not None:
        LAST_EXEC_NS = res.exec_time_ns
        if res.instructions_and_trace:
            LAST_TRACE_PATH = res.instructions_and_trace[1]
    y = np.sum(np.stack([np.asarray(res.results[c]["y"], dtype=np.float32)
                         for c in range(N_CORES)]), axis=0)
    return y.reshape(B, S, D_MODEL).astype(np.float32)


# revision 15
# speedup vs baseline: 1.8342x; 1.2059x over previous
"""Causal multi-head self-attention (RoPE) Trainium2 Bass kernel, 8-way
head-parallel, bf16 datapath.

Sharding: 16 heads / 8 cores = 2 heads per core (tensor parallel). Each core
receives the full (pre-transposed) activation matrix plus its head-slice of
w_qkv (with the RoPE interleave->half-split permutation folded into the
weight rows) and its 128-column slice of w_out. Each core computes a full
[8192, 1024] partial of the output projection; the host sums the 8 partials
(the all-reduce equivalent).

All matmul operands are bf16 (PSUM accumulation stays fp32), halving SBUF/HBM
traffic and DVE/ACT element passes vs the fp32r version.

Per core and batch:
  qkv^T = W @ x^T                 (Q,K,V chains; K accumulated in PSUM)
  u_sw  = SWAP @ u                (PE, 128x128 pair-swap matrix)
  rot   = u*cosA + u_sw*sinS      (DVE; both heads stay stacked [128, S])
  S^T   = rot_k^T rot_q, two 128-key tiles paired per PSUM [128,1024] group;
          one exp (ACT, scale=1/8) per pair -> bf16
  causal: variable-N matmuls skip above-diagonal tiles; gpsimd affine_select
          zeroes the strict lower triangle of diagonal blocks
  PV    = [V | 1]^T @ E^T         (ones column gives the softmax denominator)
  rec   = reciprocal_approx_fast(PV[64]) -> bf16; PE ones-row broadcast;
          outT = PV[0:64] * bcast(rec)  (DVE)
  y     = outT^T @ w_out_slice^T  (K=128; per-qc so it overlaps attention)
"""
import os
import sys

for _p in ("/opt/trn_rl_repo", "/root/.axon_site/_ro/trn_rl_repo"):
    if os.path.isdir(_p) and _p not in sys.path:
        sys.path.insert(0, _p)

import numpy as np
import ml_dtypes

BF = ml_dtypes.bfloat16

B, S, D_MODEL, N_HEADS, D_HEAD = 4, 2048, 1024, 16, 64
N_CORES, H_PER = 8, 2
THETA = 10000.0
BS = B * S
KC = D_MODEL // 128             # 8 contraction chunks
NQ = S // 512                   # 4 query chunks per batch
NK = S // 128                   # 16 key tiles per batch

_PERM = np.concatenate([np.arange(0, 64, 2), np.arange(1, 64, 2)])
_INVF = THETA ** (-np.arange(32) * 2.0 / 64)

_cached = {}
USE_BASE64 = True        # heads stacked in rot [128,S]; hh=1 matmuls at base 64
USE_PAIR = True          # pair two key tiles per PSUM group, one exp per pair
USE_APPROX = True        # reciprocal_approx_fast for softmax denominator
TRACE = False            # set True to capture an NTFF profile on the next run
LAST_EXEC_NS = None      # max-core HW exec time of the last traced run
LAST_TRACE_PATH = None
LAST_RESULT = None


def _host_tables():
    pos = np.arange(S, dtype=np.float64)
    ang = pos[None, :] * _INVF[:, None]
    cosb, sinb = np.cos(ang), np.sin(ang)
    cosA64 = np.concatenate([cosb, cosb], 0)
    sinS64 = np.concatenate([-sinb, sinb], 0)
    cosA = np.concatenate([cosA64, cosA64], 0).astype(BF)   # [128, S]
    sinS = np.concatenate([sinS64, sinS64], 0).astype(BF)
    sw64 = np.zeros((64, 64), np.float64)
    sw64[:32, 32:] = np.eye(32)
    sw64[32:, :32] = np.eye(32)
    SW = np.block([[sw64, np.zeros((64, 64))], [np.zeros((64, 64)), sw64]])
    return cosA, sinS, SW.astype(BF)


def _host_prep(x, w_qkv, w_out):
    cosA, sinS, SW = _host_tables()
    xT = np.ascontiguousarray(x.reshape(BS, D_MODEL).T).astype(BF)
    in_maps = []
    for c in range(N_CORES):
        h0, h1 = 2 * c, 2 * c + 1
        wq = np.concatenate([w_qkv[64 * h0:64 * h0 + 64][_PERM],
                             w_qkv[64 * h1:64 * h1 + 64][_PERM]], 0)
        wk = np.concatenate([w_qkv[1024 + 64 * h0:1024 + 64 * h0 + 64][_PERM],
                             w_qkv[1024 + 64 * h1:1024 + 64 * h1 + 64][_PERM]], 0)
        wv = np.concatenate([w_qkv[2048 + 64 * h0:2048 + 64 * h0 + 64],
                             w_qkv[2048 + 64 * h1:2048 + 64 * h1 + 64]], 0)
        w_all = np.concatenate([wq, wk, wv], 0)          # [384, 1024]
        in_maps.append({
            "xT": xT,
            "wqkvT": np.ascontiguousarray(w_all.T).astype(BF),
            "woutT": np.ascontiguousarray(
                w_out[:, 128 * c:128 * c + 128].T).astype(BF),
            "swapm": SW,
            "cosA": cosA,
            "sinS": sinS,
        })
    return in_maps


def _build_nc():
    import concourse.bacc as bacc
    import concourse.mybir as mybir
    from concourse import tile
    from concourse.masks import make_identity

    F32, BF16 = mybir.dt.float32, mybir.dt.bfloat16
    AF = mybir.ActivationFunctionType
    ALU = mybir.AluOpType

    nc = bacc.Bacc("TRN2", target_bir_lowering=False, debug=False,
                   num_devices=N_CORES)
    xT_d = nc.dram_tensor("xT", [D_MODEL, BS], BF16, kind="ExternalInput")
    w_d = nc.dram_tensor("wqkvT", [D_MODEL, 384], BF16, kind="ExternalInput")
    wo_d = nc.dram_tensor("woutT", [128, D_MODEL], BF16, kind="ExternalInput")
    sw_d = nc.dram_tensor("swapm", [128, 128], BF16, kind="ExternalInput")
    cos_d = nc.dram_tensor("cosA", [128, S], BF16, kind="ExternalInput")
    sin_d = nc.dram_tensor("sinS", [128, S], BF16, kind="ExternalInput")
    y_d = nc.dram_tensor("y", [BS, D_MODEL], BF16, kind="ExternalOutput")

    with tile.TileContext(nc) as tc:
        with tc.tile_pool(name="const", bufs=1) as const, \
             tc.tile_pool(name="xr", bufs=2) as xrp, \
             tc.tile_pool(name="ur", bufs=3) as urp, \
             tc.tile_pool(name="ropet", bufs=2) as ropet, \
             tc.tile_pool(name="rot", bufs=2) as rotp, \
             tc.tile_pool(name="vsb", bufs=2) as vsbp, \
             tc.tile_pool(name="vone", bufs=2) as vonep, \
             tc.tile_pool(name="ep", bufs=4) as ep, \
             tc.tile_pool(name="recp", bufs=2) as recp, \
             tc.tile_pool(name="outp", bufs=2) as outp, \
             tc.tile_pool(name="ysb", bufs=3) as ysb, \
             tc.tile_pool(name="ps", bufs=1, space="PSUM") as ps:

            # ---- constants -------------------------------------------------
            w_r = const.tile([128, KC, 384], BF16, tag="w")
            nc.sync.dma_start(
                w_r[:], w_d.ap().rearrange("(kc p) m -> p kc m", p=128))
            wo_r = const.tile([128, D_MODEL], BF16, tag="wo")
            nc.sync.dma_start(wo_r[:], wo_d.ap())
            swap_r = const.tile([128, 128], BF16, tag="swap")
            nc.sync.dma_start(swap_r[:], sw_d.ap())
            cosA = const.tile([128, S], BF16, tag="cos")
            sinS = const.tile([128, S], BF16, tag="sin")
            nc.sync.dma_start(cosA[:], cos_d.ap())
            nc.sync.dma_start(sinS[:], sin_d.ap())
            idf = const.tile([128, 128], F32, tag="idf")
            make_identity(nc, idf[:])
            id_b = const.tile([128, 128], BF16, tag="idb")
            nc.vector.tensor_copy(id_b[:], idf[:])
            onef = const.tile([128, 1], F32, tag="onef")
            nc.vector.memset(onef[:], 1.0)
            one_b = const.tile([128, 1], BF16, tag="oneb")
            nc.vector.tensor_copy(one_b[:], onef[:])
            orow_f = const.tile([1, 64], F32, tag="orowf")
            nc.vector.memset(orow_f[:], 1.0)
            orow_b = const.tile([1, 64], BF16, tag="orow")
            nc.vector.tensor_copy(orow_b[:], orow_f[:])

            for b in range(B):
                c0 = b * S

                # ---- QKV projection + RoPE ---------------------------------
                if USE_BASE64:
                    rot_q = rotp.tile([128, S], BF16, tag="rq")
                    rot_k = rotp.tile([128, S], BF16, tag="rk")
                else:
                    rot_q = rotp.tile([64, 2 * S], BF16, tag="rq")
                    rot_k = rotp.tile([64, 2 * S], BF16, tag="rk")

                def rsl(rot, hh, start, length):
                    if USE_BASE64:
                        return rot[64 * hh:64 * hh + 64, start:start + length]
                    return rot[:, hh * S + start:hh * S + start + length]
                v_sb = vsbp.tile([128, S], BF16, tag="vsb")
                vone = vonep.tile([128, H_PER, NK, 65], BF16, tag="vone")
                for hh in range(H_PER):
                    for ki in range(NK):
                        nc.gpsimd.tensor_copy(vone[:, hh, ki, 64:65], one_b[:])

                for n in range(NQ):
                    nsl = slice(n * 512, (n + 1) * 512)
                    x_r = xrp.tile([128, KC, 512], BF16, tag="xr")
                    for half in range(2):
                        nc.sync.dma_start(
                            x_r[:, 4 * half:4 * half + 4, :],
                            xT_d.ap()[512 * half:512 * half + 512,
                                      c0 + n * 512:c0 + (n + 1) * 512]
                            .rearrange("(kc p) n -> p kc n", p=128))

                    if USE_PAIR:
                        tqk = ps.tile([128, 1024], F32, tag="a", bufs=2)
                        halves = [tqk[:, :512], tqk[:, 512:]]
                    else:
                        t0 = ps.tile([128, 512], F32, tag="a", bufs=4)
                        t1 = ps.tile([128, 512], F32, tag="a", bufs=4)
                        halves = [t0[:], t1[:]]
                    for qk in range(2):        # 0: Q, 1: K
                        for kc in range(KC):
                            nc.tensor.matmul(
                                halves[qk],
                                w_r[:, kc, 128 * qk:128 * qk + 128],
                                x_r[:, kc, :], start=(kc == 0),
                                stop=(kc == KC - 1))
                    pvv = ps.tile([128, 512], F32, tag="v", bufs=2)
                    for kc in range(KC):
                        nc.tensor.matmul(pvv[:], w_r[:, kc, 256:384],
                                         x_r[:, kc, :], start=(kc == 0),
                                         stop=(kc == KC - 1))
                    uq = urp.tile([128, 512], BF16, tag="ur")
                    nc.scalar.copy(uq[:], halves[0])
                    uk = urp.tile([128, 512], BF16, tag="ur")
                    nc.scalar.copy(uk[:], halves[1])
                    nc.scalar.copy(v_sb[:, nsl], pvv[:])
                    if USE_PAIR:
                        tsw = ps.tile([128, 1024], F32, tag="a", bufs=2)
                        swh = [tsw[:, :512], tsw[:, 512:]]
                    else:
                        s0 = ps.tile([128, 512], F32, tag="a", bufs=4)
                        s1 = ps.tile([128, 512], F32, tag="a", bufs=4)
                        swh = [s0[:], s1[:]]
                    nc.tensor.matmul(swh[0], swap_r[:], uq[:],
                                     start=True, stop=True)
                    nc.tensor.matmul(swh[1], swap_r[:], uk[:],
                                     start=True, stop=True)
                    for qk, (u_sb, rot) in enumerate(((uq, rot_q),
                                                      (uk, rot_k))):
                        m_sb = ropet.tile([128, 512], BF16, tag="m")
                        nc.vector.tensor_mul(m_sb[:], u_sb[:], cosA[:, nsl])
                        t_sb = ropet.tile([128, 512], BF16, tag="t")
                        nc.vector.tensor_mul(
                            t_sb[:], swh[qk], sinS[:, nsl])
                        if USE_BASE64:
                            nc.vector.tensor_add(rot[:, nsl], m_sb[:],
                                                 t_sb[:])
                        else:
                            for hh in range(H_PER):
                                nc.vector.tensor_add(
                                    rsl(rot, hh, n * 512, 512),
                                    m_sb[64 * hh:64 * hh + 64, :],
                                    t_sb[64 * hh:64 * hh + 64, :])

                # ---- V transpose ------------------------------------------
                for ki in range(NK):
                    pt = ps.tile([128, 128], BF16, tag="v", bufs=2)
                    nc.tensor.transpose(
                        pt[:], v_sb[:, ki * 128:(ki + 1) * 128], id_b[:])
                    nc.vector.tensor_copy(
                        vone[:, :, ki, :64],
                        pt[:].rearrange("p (h d) -> p h d", h=2))

                # ---- attention + output projection, per query chunk -------
                outT = outp.tile([128, S], BF16, tag="outT")
                step = 2 if USE_PAIR else 1
                stw = 1024 if USE_PAIR else 512
                stb = 2 if USE_PAIR else 4
                for qc in range(NQ):
                    last_ki = 4 * qc + 3
                    pvs = [ps.tile([65, 512], F32, tag="pv", bufs=2,
                                   name=f"pv{hh}") for hh in range(H_PER)]
                    # pair key tiles: (0,1), (2,3), ...; interleave the two
                    # heads' S matmuls so their K=64 tiles (array rows 0-63
                    # vs 64-127) run concurrently on the PE.
                    pend = []   # (hh, e_t, [(ki, ncc, off)]) awaiting PV
                    def pop_pv():
                        hh_, pe_t, pgroup = pend.pop(0)
                        for ki, ncc, o in pgroup:
                            nc.tensor.matmul(
                                pvs[hh_][:, 512 - ncc:],
                                vone[:, hh_, ki, :], pe_t[:, o:o + ncc],
                                start=(ki == 0), stop=(ki == last_ki))
                    for k0 in range(0, last_ki + 1, step):
                        group = []
                        off = 0
                        for ki in range(k0, k0 + step):
                            if ki > last_ki:
                                break
                            ncc = 512 - max(0, ki - 4 * qc) * 128
                            group.append((ki, ncc, off))
                            off += ncc
                        stps = [ps.tile([128, stw], F32, tag="a", bufs=stb,
                                        name=f"st{hh}") for hh in range(H_PER)]
                        e_ts = [ep.tile([128, stw], BF16, tag="e",
                                        name=f"et{hh}") for hh in range(H_PER)]
                        for ki, ncc, o in group:
                            coff = max(qc * 512, ki * 128)
                            for hh in range(H_PER):
                                nc.tensor.matmul(
                                    stps[hh][:, o:o + ncc],
                                    rsl(rot_k, hh, ki * 128, 128),
                                    rsl(rot_q, hh, coff, ncc),
                                    start=True, stop=True)
                        for hh in range(H_PER):
                            nc.scalar.activation(e_ts[hh][:, :off],
                                                 stps[hh][:, :off],
                                                 AF.Exp, scale=0.125)
                            for ki, ncc, o in group:
                                if ki >= 4 * qc:   # diagonal block: zero k > q
                                    nc.gpsimd.affine_select(
                                        out=e_ts[hh][:, o:o + 128],
                                        in_=e_ts[hh][:, o:o + 128],
                                        compare_op=ALU.is_ge, fill=0.0,
                                        base=0, pattern=[[1, 128]],
                                        channel_multiplier=-1)
                            pend.append((hh, e_ts[hh], group))
                        while len(pend) > 2:
                            pop_pv()
                    while pend:
                        pop_pv()

                    for hh in range(H_PER):
                        rec_f = recp.tile([1, 512], F32, tag="rec")
                        if USE_APPROX:
                            den_sb = recp.tile([1, 512], F32, tag="den")
                            nc.scalar.copy(den_sb[:], pvs[hh][64:65, :])
                            nc.vector.reciprocal_approx_fast(rec_f[:],
                                                             den_sb[:])
                        else:
                            nc.vector.reciprocal(rec_f[:], pvs[hh][64:65, :])
                        rec_b = recp.tile([1, 512], BF16, tag="recb")
                        nc.vector.tensor_copy(rec_b[:], rec_f[:])
                        pbc = ps.tile([64, 512], F32, tag="v", bufs=2)
                        nc.tensor.matmul(pbc[:], orow_b[:], rec_b[:],
                                         start=True, stop=True)
                        rb_sb = recp.tile([64, 512], BF16, tag="rb")
                        nc.scalar.copy(rb_sb[:], pbc[:])
                        nc.vector.tensor_mul(
                            outT[64 * hh:64 * hh + 64,
                                 qc * 512:(qc + 1) * 512],
                            pvs[hh][:64, :], rb_sb[:])

                    # ---- output projection for this qc's token tiles ------
                    for t in range(4 * qc, 4 * qc + 4):
                        y_sb = ysb.tile([128, 1024], BF16, tag="ysb")
                        for ec in range(2):
                            py = ps.tile([128, 512], F32, tag="pv", bufs=2)
                            nc.tensor.matmul(py[:],
                                             outT[:, t * 128:(t + 1) * 128],
                                             wo_r[:, ec * 512:(ec + 1) * 512],
                                             start=True, stop=True)
                            nc.vector.tensor_copy(
                                y_sb[:, ec * 512:(ec + 1) * 512], py[:])
                        nc.sync.dma_start(
                            y_d.ap()[c0 + t * 128: c0 + (t + 1) * 128, :],
                            y_sb[:])
    nc.compile()
    return nc


def _get_nc():
    if "nc" not in _cached:
        _cached["nc"] = _build_nc()
    return _cached["nc"]


def kernel(x, w_qkv, w_out):
    from concourse.bass_utils import run_bass_kernel_spmd

    x = np.asarray(x, np.float32)
    w_qkv = np.asarray(w_qkv, np.float32)
    w_out = np.asarray(w_out, np.float32)
    in_maps = _host_prep(x, w_qkv, w_out)
    nc = _get_nc()
    res = run_bass_kernel_spmd(nc, in_maps, core_ids=list(range(N_CORES)),
                               trace=TRACE)
    global LAST_EXEC_NS, LAST_TRACE_PATH, LAST_RESULT
    LAST_RESULT = res
    if res.exec_time_ns is not None:
        LAST_EXEC_NS = res.exec_time_ns
        if res.instructions_and_trace:
            LAST_TRACE_PATH = res.instructions_and_trace[1]
    y = np.sum(np.stack([np.asarray(res.results[c]["y"], dtype=np.float32)
                         for c in range(N_CORES)]), axis=0)
    return y.reshape(B, S, D_MODEL).astype(np.float32)
